# revision 2
# baseline (speedup 1.0000x reference)
"""AugmentPipe Trainium2 kernel v4.

Rot path: work unit = 8 consecutive 32x32 blocks (half a block-row) of one
rotated sample; 32R/8 = 4R units per core, exactly balanced. Per unit:
bf16 crop planes with 12 shifted variants (3ch x dy{0,1} x parity{0,1})
DMA'd densely; two 512-idx ap_gathers (d=2 bf16 pair fetch; ~3.2us each on
the Q7s vs 22us for one 1024-idx gather) fetch all 4 bilinear taps x 3
channels per pixel; DVE multiplies by bf16 weights (parity-selected,
contrast-prescaled); PE reduces partitions via a shifted-window 0/1
stationary into a PSUM tile batching 16 units; fused post (brightness/
clip/saturation/cutout) runs on all 128 partitions; output written as a
dense per-unit tile dump, final block placement done on host (pure layout).

Axis path (m_rot=0): separable one-hot V/H matmuls in bf16, direct output.
"""

import numpy as np

BF16 = np.dtype(np.float16)

B, C, H, W = 64, 3, 512, 512
NCORES = 8
BLK = 32
GRID = H // BLK            # 16 blocks per row
UPS = GRID * GRID // 8     # 32 units per sample
NI = BLK * BLK             # 1024 px per block
GU = 16                    # units per psum post group
import os as _os
SUBN = int(_os.environ.get('K4SUBN', '512'))   # idx per ap_gather call
NV = 12                    # shipped plane variants per block (16 = debug/pad)

# px gather order: stride-4 interleave so each 4-idx read request hits
# distant plane addresses (avoids SBUF bank conflicts on clustered taps)
PERM = ((np.arange(NI) % 4) * (NI // 4) + np.arange(NI) // 4).astype(np.int64)

TRANSLATE_STD = np.float32(0.125)
SCALE_STD = np.float32(0.2)

_PROGRAM_CACHE = {}


# ---------------------------------------------------------------- host math
def _host_taps(inputs):
    f = np.float32
    u_angle = inputs['u_angle'].astype(f); u_scale = inputs['u_scale'].astype(f)
    u_trans = inputs['u_trans'].astype(f)
    m_rot = inputs['m_rot']; m_scale = inputs['m_scale']; m_trans = inputs['m_trans']
    m_flip = inputs['m_flip']

    angle = np.where(m_rot > 0, (u_angle * f(2.0) - f(1.0)) * f(np.pi), f(0.0)).astype(f)
    sc = np.where(m_scale > 0, (u_scale * f(2.0) - f(1.0)) * SCALE_STD + f(1.0), f(1.0)).astype(f)
    tr = np.where(m_trans > 0, (u_trans * f(2.0) - f(1.0)) * TRANSLATE_STD, f(0.0)).astype(f)
    ca = np.cos(angle).astype(f); sa = np.sin(angle).astype(f)

    lin = np.linspace(f(-1.0), f(1.0), W, dtype=f)
    gx0, gy0 = np.meshgrid(lin, lin, indexing='xy')

    out = []
    for b in range(B):
        gx = (sc[b] * (ca[b] * gx0 - sa[b] * gy0) + tr[b]).astype(f)
        gy = (sc[b] * (sa[b] * gx0 + ca[b] * gy0) + tr[b]).astype(f)
        x = ((gx + f(1.0)) * f(W) - f(1.0)) * f(0.5)
        y = ((gy + f(1.0)) * f(H) - f(1.0)) * f(0.5)

        def reflect(v, size):
            v = np.abs(v + f(0.5))
            v = np.mod(v, f(2.0 * size))
            v = np.minimum(v, f(2.0 * size) - v)
            return np.clip(v - f(0.5), f(0.0), f(size - 1.0)).astype(f)

        x = reflect(x, float(W)); y = reflect(y, float(H))
        x0f = np.floor(x); y0f = np.floor(y)
        wx = (x - x0f).astype(f); wy = (y - y0f).astype(f)
        x0 = np.clip(x0f, 0, W - 1).astype(np.int32)
        x1 = np.clip(x0f + 1, 0, W - 1).astype(np.int32)
        y0 = np.clip(y0f, 0, H - 1).astype(np.int32)
        y1 = np.clip(y0f + 1, 0, H - 1).astype(np.int32)
        if m_flip[b] > 0:
            x0 = W - 1 - x0
            x1 = W - 1 - x1
        out.append((y0, y1, x0, x1, wy, wx))
    return out


def _axis_matrices(tap):
    y0, y1, x0, x1, wy, wx = tap
    f = np.float32
    Wv = np.zeros((H, H), f)
    r_i = np.arange(H)
    np.add.at(Wv, (r_i, y0[:, 0]), (f(1.0) - wy[:, 0]))
    np.add.at(Wv, (r_i, y1[:, 0]), wy[:, 0])
    Wh = np.zeros((W, W), f)
    np.add.at(Wh, (x0[0, :], r_i), (f(1.0) - wx[0, :]))
    np.add.at(Wh, (x1[0, :], r_i), wx[0, :])
    return np.ascontiguousarray(Wv.T), Wh


def _unit_geom(tap, t):
    """Crop extents + flat pair indices + tap weights for unit t (blocks
    8t..8t+7)."""
    y0, y1, x0, x1, wy, wx = tap
    f = np.float32
    c_lo = np.minimum(x0, x1)
    wcl = np.where(x1 == x0, f(1.0),
                   np.where(x1 > x0, f(1.0) - wx, wx)).astype(f)
    wcr = np.where(x1 == x0, f(0.0),
                   np.where(x1 > x0, wx, f(1.0) - wx)).astype(f)
    wrt = np.where(y1 == y0, f(1.0), f(1.0) - wy).astype(f)
    wrb = np.where(y1 == y0, f(0.0), wy).astype(f)

    blocks = []
    for g in range(8):
        b = t * 8 + g
        bi, bj = b // GRID, b % GRID
        sl = (slice(bi * BLK, (bi + 1) * BLK), slice(bj * BLK, (bj + 1) * BLK))
        by0 = y0[sl].ravel(); by1 = y1[sl].ravel()
        bcl = c_lo[sl].ravel()
        r0 = int(by0.min()); c0 = int(bcl.min())
        rh = int(by1.max()) - r0 + 1
        cw = int(bcl.max()) - c0 + 2
        fidx = (by0.astype(np.int64) - r0) * cw + (bcl.astype(np.int64) - c0)
        fidx = fidx[PERM]
        blocks.append(dict(
            r0=r0, c0=c0, rh=rh, cw=cw,
            k=(fidx >> 1).astype(np.int16), par=(fidx & 1).astype(np.int64),
            wrt=wrt[sl].ravel()[PERM], wrb=wrb[sl].ravel()[PERM],
            wcl=wcl[sl].ravel()[PERM], wcr=wcr[sl].ravel()[PERM],
        ))
    need = max((bl['rh'] + 1) * bl['cw'] + 2 for bl in blocks)
    return blocks, -(-need // 2)      # plane size in u32 pairs


def _host_prep(inputs):
    f = np.float32
    taps = _host_taps(inputs)
    m_rot = np.asarray(inputs['m_rot'])
    rot_s = [s for s in range(B) if m_rot[s] > 0]
    ax_s = [s for s in range(B) if m_rot[s] <= 0]
    R = len(rot_s)

    u_b = inputs['u_bright'].astype(f); u_c = inputs['u_contrast'].astype(f)
    u_s = inputs['u_sat'].astype(f)
    bb = np.where(inputs['m_bright'] > 0, u_b * f(0.2), f(0.0)).astype(f)
    cc = np.where(inputs['m_contrast'] > 0, u_c + f(0.5), f(1.0)).astype(f)
    ss = np.where(inputs['m_sat'] > 0, u_s * f(2.0), f(1.0)).astype(f)
    y0c = np.asarray(inputs['y0']); x0c = np.asarray(inputs['x0'])
    m_cut = np.asarray(inputs['m_cut'])
    images = np.asarray(inputs['images'], dtype=f)
    noise = np.asarray(inputs['noise'], dtype=f)

    NRU = 4 * R
    NG = -(-NRU // GU) if NRU else 0
    NAXR = len(ax_s)
    NAX = -(-NAXR // NCORES) if NAXR else 0

    # ---------- per-core unit lists ----------
    all_units = [(s, t) for s in rot_s for t in range(UPS)]
    ys = np.arange(H); xs = np.arange(W)

    def unit_cut(s, t):
        if m_cut[s] <= 0:
            return False
        yy, xx = int(y0c[s]), int(x0c[s])
        bi_lo = t * 8 // GRID
        for g in range(8):
            b = t * 8 + g
            bi, bj = b // GRID, b % GRID
            if (bi * BLK < yy + 256 and (bi + 1) * BLK > yy and
                    bj * BLK < xx + 256 and (bj + 1) * BLK > xx):
                return True
        return False

    meta = []
    for c in range(NCORES):
        lst = []
        for (s, t) in all_units[c * NRU:(c + 1) * NRU]:
            blocks, pl = _unit_geom(taps[s], t)
            lst.append(dict(s=s, t=t, blocks=blocks, plane=pl,
                            cut=unit_cut(s, t)))
        lst.sort(key=lambda u: (not u['cut'], -u['plane']))
        meta.append(lst)

    plane_u = [max(meta[c][i]['plane'] for c in range(NCORES))
               for i in range(NRU)]
    grp_cut = [any(meta[c][gi * GU + j]['cut']
                   for c in range(NCORES)
                   for j in range(min(GU, NRU - gi * GU)))
               for gi in range(NG)]

    pat_off = [0]
    for p in plane_u:
        pat_off.append(pat_off[-1] + 8 * NV * 2 * p)
    PATN = max(pat_off[-1], 2)

    # stationary: shifted-window trick. Sfull [128, 248]; slice
    # [:, 120-8j : 248-8j] puts unit j's 8 columns at psum rows 8j..8j+8.
    # col q of slice -> Sfull col q+120-8j; nonzero cols 120..128 of Sfull.
    # Sfull[:, 120+g] has 1s at rows 16g+4*c3+v -- but c3 varies per chain!
    # -> separate Sfull per c3: [128, 3, 248].
    Sfull = np.zeros((128, 3, 248), f)
    for c3 in range(3):
        for g in range(8):
            for v in range(4):
                Sfull[16 * g + 4 * c3 + v, c3, 120 + g] = 1.0

    # axis: per-core sample lists, cut-first, padded with duplicates
    ax_lists = []
    for c in range(NCORES):
        lst = [ax_s[k] for k in range(c, NAXR, NCORES)]
        lst.sort(key=lambda s: m_cut[s] <= 0)
        own = len(lst)
        while len(lst) < NAX:
            lst.append(lst[-1] if lst else 0)
        ax_lists.append((lst, own))
    ax_cut = [any(m_cut[ax_lists[c][0][k]] > 0 for c in range(NCORES))
              if NAX else False for k in range(NAX)]

    cores = []
    for c in range(NCORES):
        lst = meta[c]
        pat = np.zeros(PATN, BF16)
        w4 = np.zeros((max(NRU, 1), 128, 2 * NI), BF16)
        idxT = np.zeros((128, max(NRU, 1), NI // 16), np.int16)
        nzm = np.zeros((max(NG, 1), 128, 3 * NI), f)
        mk1 = np.ones((max(NG, 1), 128, 3 * NI), f)
        scal = np.zeros((128, max(NG, 1), 8), f)
        outmap = []
        for i, u in enumerate(lst):
            s, t = u['s'], u['t']
            outmap.append((s, t))
            gi, j = i // GU, i % GU
            ccs = cc[s]
            pl2 = 2 * plane_u[i]
            ubase = pat_off[i]
            cut_on = m_cut[s] > 0
            if cut_on:
                yy, xx = int(y0c[s]), int(x0c[s])
            for g, bl in enumerate(u['blocks']):
                r0, c0, rh, cw = bl['r0'], bl['c0'], bl['rh'], bl['cw']
                hh = min(rh + 1, H - r0); ww = min(cw, W - c0)
                flat = np.zeros(pl2 + cw + 2, f)
                par = bl['par']; k = bl['k']
                # weights: w4[p=16g+4c4+2dy+par', 2px+dx]
                wrow = np.empty((2, 2, NI, 2), f)   # [dy, par', px, dx]
                for dy in range(2):
                    wy_ = bl['wrt'] if dy == 0 else bl['wrb']
                    for pr in range(2):
                        sel = (par == pr).astype(f)
                        wrow[dy, pr, :, 0] = wy_ * bl['wcl'] * sel * ccs
                        wrow[dy, pr, :, 1] = wy_ * bl['wcr'] * sel * ccs
                wq = wrow.reshape(4, 2 * NI).astype(BF16)
                for c4 in range(NV // 4):
                    crop = np.zeros((rh + 1, cw), f)
                    crop[:hh, :ww] = images[s, min(c4, 2), r0:r0 + hh, c0:c0 + ww]
                    flat[:(rh + 1) * cw] = crop.ravel()
                    fb = flat.astype(BF16)
                    for dy in range(2):
                        for pr in range(2):
                            v = 4 * c4 + 2 * dy + pr
                            st = dy * cw + pr
                            dst = ubase + (g * NV + v) * pl2
                            pat[dst:dst + pl2] = fb[st:st + pl2]
                    if c4 < 3:
                        w4[i, 16 * g + 4 * c4:16 * g + 4 * c4 + 4, :] = wq
                # idx: item n -> (partition 16g + n%16, col n//16)
                idxT[16 * g:16 * g + 16, i, :] = k.reshape(NI // 16, 16).T
            p_lo, p_hi = 8 * j, 8 * j + 8
            m = min(float(cc[s]), 1.0)
            scal[p_lo:p_hi, gi, 1] = ccs * bb[s]
            scal[p_lo:p_hi, gi, 2] = m
            scal[p_lo:p_hi, gi, 5] = -m
            scal[p_lo:p_hi, gi, 3] = ss[s]
            scal[p_lo:p_hi, gi, 4] = (f(1.0) - ss[s]) / f(3.0)
            if cut_on and grp_cut[gi]:
                rmv = (ys >= yy) & (ys < yy + 256)
                cmv = (xs >= xx) & (xs < xx + 256)
                for g in range(8):
                    b8 = t * 8 + g
                    bi, bj = b8 // GRID, b8 % GRID
                    m2 = (rmv[bi * BLK:(bi + 1) * BLK, None] &
                          cmv[None, bj * BLK:(bj + 1) * BLK]).astype(f)
                    mflat = m2.ravel()[PERM]
                    for c3 in range(3):
                        nz = noise[s, c3, bi * BLK:(bi + 1) * BLK,
                                   bj * BLK:(bj + 1) * BLK].ravel()[PERM]
                        nzm[gi, p_lo + g, c3 * NI:(c3 + 1) * NI] = nz * mflat
                        mk1[gi, p_lo + g, c3 * NI:(c3 + 1) * NI] = f(1.0) - mflat

        # ---------------- axis tensors ----------------
        axl, axown = ax_lists[c]
        wvT = np.zeros((max(NAX, 1), H, H), BF16)
        wh = np.zeros((max(NAX, 1), W, W), BF16)
        imga = np.zeros((max(NAX, 1), C, H, W), BF16)
        nza = np.zeros((max(NAX, 1), C, H, W), f)
        cm = np.zeros((max(NAX, 1), 128, W), f)
        rm = np.zeros((max(NAX, 1), 128, 4), f)
        scax = np.zeros((128, max(NAX, 1), 8), f)
        for k2, s in enumerate(axl):
            wv_, wh_ = _axis_matrices(taps[s])
            wvT[k2] = wv_.astype(BF16)
            wh[k2] = wh_.astype(BF16)
            imga[k2] = images[s].astype(BF16)
            nza[k2] = noise[s]
            m = min(float(cc[s]), 1.0)
            scax[:, k2, 0] = cc[s]; scax[:, k2, 1] = cc[s] * bb[s]
            scax[:, k2, 2] = m; scax[:, k2, 5] = -m
            scax[:, k2, 3] = ss[s]; scax[:, k2, 4] = (f(1.0) - ss[s]) / f(3.0)
            if m_cut[s] > 0:
                cmv = np.zeros(W, f); cmv[x0c[s]:x0c[s] + 256] = 1.0
                rmv = np.zeros(H, f); rmv[y0c[s]:y0c[s] + 256] = 1.0
                cm[k2] = cmv[None, :]
                rm[k2] = rmv.reshape(4, 128).T

        cores.append(dict(
            pat=pat, w4=w4, idx=idxT, nzm=nzm, mk1=mk1, scal=scal,
            sfull=Sfull.reshape(128, 3 * 248).astype(BF16),
            wvT=wvT, wh=wh, imga=imga, nza=nza, cm=cm, rm=rm, scax=scax,
            ident=np.eye(128, dtype=f),
            outmap=outmap,
        ))

    plan = dict(NRU=NRU, NG=NG, NAX=NAX, PATN=PATN,
                plane_u=tuple(plane_u), pat_off=tuple(pat_off),
                grp_cut=tuple(grp_cut), ax_cut=tuple(ax_cut))
    axinfo = [ax_lists[c] for c in range(NCORES)]
    return cores, [m_ for m_ in meta], axinfo, plan


# ---------------------------------------------------------------- device
def _build(plan):
    import os
    ABL = set(os.environ.get('K4ABL', '').split(','))
    import concourse.bacc as bacc
    import concourse.mybir as mybir
    from concourse import tile

    NRU = plan['NRU']; NG = plan['NG']; NAX = plan['NAX']
    NV_ = NV
    plane_u = plan['plane_u']; pat_off = plan['pat_off']
    grp_cut = plan['grp_cut']; ax_cut = plan['ax_cut']
    PATN = plan['PATN']
    plane_max = max(plane_u) if plane_u else 1

    f32 = mybir.dt.float32
    bf16 = mybir.dt.float16
    i16 = mybir.dt.int16
    nc = bacc.Bacc()

    d = {}
    d['pat'] = nc.dram_tensor('pat', [PATN], bf16, kind='ExternalInput')
    d['w4'] = nc.dram_tensor('w4', [max(NRU, 1), 128, 2 * NI], bf16, kind='ExternalInput')
    d['idx'] = nc.dram_tensor('idx', [128, max(NRU, 1), NI // 16], i16, kind='ExternalInput')
    d['nzm'] = nc.dram_tensor('nzm', [max(NG, 1), 128, 3 * NI], f32, kind='ExternalInput')
    d['mk1'] = nc.dram_tensor('mk1', [max(NG, 1), 128, 3 * NI], f32, kind='ExternalInput')
    d['scal'] = nc.dram_tensor('scal', [128, max(NG, 1), 8], f32, kind='ExternalInput')
    d['sfull'] = nc.dram_tensor('sfull', [128, 3 * 248], bf16, kind='ExternalInput')
    d['wvT'] = nc.dram_tensor('wvT', [max(NAX, 1), H, H], bf16, kind='ExternalInput')
    d['wh'] = nc.dram_tensor('wh', [max(NAX, 1), W, W], bf16, kind='ExternalInput')
    d['imga'] = nc.dram_tensor('imga', [max(NAX, 1), C, H, W], bf16, kind='ExternalInput')
    d['nza'] = nc.dram_tensor('nza', [max(NAX, 1), C, H, W], f32, kind='ExternalInput')
    d['cm'] = nc.dram_tensor('cm', [max(NAX, 1), 128, W], f32, kind='ExternalInput')
    d['rm'] = nc.dram_tensor('rm', [max(NAX, 1), 128, 4], f32, kind='ExternalInput')
    d['scax'] = nc.dram_tensor('scax', [128, max(NAX, 1), 8], f32, kind='ExternalInput')
    d['ident'] = nc.dram_tensor('ident', [128, 128], f32, kind='ExternalInput')
    d_dump = nc.dram_tensor('dump', [max(NRU, 1), 8, 3 * NI], f32, kind='ExternalOutput')
    d_oax = nc.dram_tensor('oax', [max(NAX, 1), C, H, W], f32, kind='ExternalOutput')

    mult = mybir.AluOpType.mult
    add = mybir.AluOpType.add
    amin = mybir.AluOpType.min
    amax = mybir.AluOpType.max

    with tile.TileContext(nc) as tc:
        with (
            tc.tile_pool(name='const', bufs=1) as cpool,
            tc.tile_pool(name='plane', bufs=3) as plpool,
            tc.tile_pool(name='gw', bufs=3) as gwpool,
            tc.tile_pool(name='post', bufs=2) as ppool,
            tc.tile_pool(name='ax', bufs=1) as apool,
            tc.tile_pool(name='rpsum', bufs=1, space='PSUM') as rpspool,
            tc.tile_pool(name='apsum', bufs=2, space='PSUM') as apspool,
        ):
            sf_sb = cpool.tile([128, 3 * 248], bf16, tag='sf')
            nc.sync.dma_start(sf_sb[:], d['sfull'][:])
            sc_sb = cpool.tile([128, max(NG, 1), 8], f32, tag='sc')
            nc.sync.dma_start(sc_sb[:], d['scal'][:])
            ident = cpool.tile([128, 128], f32, tag='id')
            nc.sync.dma_start(ident[:], d['ident'][:])
            scx_sb = cpool.tile([128, max(NAX, 1), 8], f32, tag='scx')
            nc.sync.dma_start(scx_sb[:], d['scax'][:])
            if NRU:
                ix_sb = cpool.tile([128, NRU, NI // 16], i16, tag='ix')
                nc.sync.dma_start(ix_sb[:], d['idx'][:, 0:NRU, :])

            # ---------------- rotated units ----------------
            for gi in range(NG):
                n_in_g = min(GU, NRU - gi * GU)
                ps = rpspool.tile([128, 3 * NI], f32, tag='ps')
                for j in range(n_in_g):
                    i = gi * GU + j
                    pl2 = 2 * plane_u[i]
                    P = plpool.tile([128, 2 * plane_max], bf16, tag='P')
                    if 'nopat' not in ABL:
                        nc.sync.dma_start(
                            P[:, 0:pl2],
                            d['pat'][pat_off[i]:pat_off[i] + 128 * pl2].rearrange(
                                "(p e) -> p e", p=128))
                    else:
                        nc.sync.dma_start(P[:, 0:4], d['pat'][0:512].rearrange("(p e) -> p e", p=128))
                    WT = gwpool.tile([128, 2 * NI], bf16, tag='WT')
                    if 'now4' not in ABL:
                        nc.scalar.dma_start(WT[:], d['w4'][i])
                    else:
                        nc.scalar.dma_start(WT[:, 0:4], d['w4'][i, :, 0:4])
                    G = gwpool.tile([128, 2 * NI], bf16, tag='G')
                    if 'nogather' not in ABL:
                        for k in range(NI // SUBN):
                            nc.gpsimd.ap_gather(
                                G[:, 2 * k * SUBN:2 * (k + 1) * SUBN].rearrange(
                                    "p (n i) -> p n i", i=2),
                                P[:, 0:pl2].rearrange("p (n i) -> p n i", i=2),
                                ix_sb[:, i, k * (SUBN // 16):(k + 1) * (SUBN // 16)],
                                channels=128, num_elems=plane_u[i], d=2,
                                num_idxs=SUBN)
                    else:
                        nc.vector.memset(G[:, 0:4], 0.5)
                    if 'nomult' not in ABL:
                        nc.vector.tensor_tensor(G[:], G[:], WT[:], op=mult)
                    if 'nomm' not in ABL:
                        for c3 in range(3):
                            for hf in range(2):
                                col = c3 * NI + hf * 512
                                for dx in range(2):
                                    nc.tensor.matmul(
                                        ps[:, col:col + 512],
                                        sf_sb[:, c3 * 248 + 120 - 8 * j:
                                              c3 * 248 + 248 - 8 * j],
                                        G[:].rearrange("p (x i) -> p x i", i=2)[
                                            :, hf * 512:hf * 512 + 512, dx],
                                        start=(j == 0 and dx == 0),
                                        stop=(j == n_in_g - 1 and dx == 1))
                    elif j == 0:
                        nc.tensor.matmul(ps[:, 0:512], sf_sb[:, 120:248],
                                         G[:, 0:512], start=True, stop=True)
                # ---- fused post on the whole group ----
                X = ppool.tile([128, 3 * NI], f32, tag='X')
                nc.vector.tensor_scalar(
                    X[:], ps[:], sc_sb[:, gi, 1:2], sc_sb[:, gi, 2:3],
                    op0=add, op1=amin)
                nc.vector.tensor_scalar(
                    X[:], X[:], sc_sb[:, gi, 5:6], None, op0=amax)
                gray = ppool.tile([128, NI], f32, tag='gray')
                nc.vector.tensor_tensor(gray[:], X[:, 0:NI], X[:, NI:2 * NI], op=add)
                nc.vector.tensor_tensor(gray[:], gray[:], X[:, 2 * NI:3 * NI], op=add)
                nc.vector.tensor_scalar(gray[:], gray[:], sc_sb[:, gi, 4:5],
                                        None, op0=mult)
                for c3 in range(3):
                    nc.vector.scalar_tensor_tensor(
                        X[:, c3 * NI:(c3 + 1) * NI], X[:, c3 * NI:(c3 + 1) * NI],
                        sc_sb[:, gi, 3:4], gray[:], op0=mult, op1=add)
                nc.vector.tensor_scalar(X[:], X[:], 1.0, -1.0, op0=amin, op1=amax)
                if grp_cut[gi]:
                    MK = ppool.tile([128, 3 * NI], f32, tag='MK')
                    NZ = ppool.tile([128, 3 * NI], f32, tag='NZ')
                    nc.scalar.dma_start(MK[:], d['mk1'][gi])
                    nc.scalar.dma_start(NZ[:], d['nzm'][gi])
                    nc.vector.tensor_tensor(X[:], X[:], MK[:], op=mult)
                    nc.vector.tensor_tensor(X[:], X[:], NZ[:], op=add)
                nc.sync.dma_start(
                    d_dump[gi * GU:gi * GU + n_in_g].rearrange("u g n -> (u g) n"),
                    X[0:8 * n_in_g, :])

            # ---------------- axis samples ----------------
            for ka in range(NAX):
                wv_sb = apool.tile([128, 4, H], bf16, tag='wv')
                wh_sb = apool.tile([128, 4, W], bf16, tag='wh')
                nc.sync.dma_start(wv_sb[:], d['wvT'][ka].rearrange("(t p) i -> p t i", p=128))
                nc.sync.dma_start(wh_sb[:], d['wh'][ka].rearrange("(t p) j -> p t j", p=128))
                Ot = []
                for ch in range(C):
                    img_sb = apool.tile([128, 4, W], bf16, tag='img')
                    nc.sync.dma_start(img_sb[:], d['imga'][ka, ch].rearrange("(t p) c -> p t c", p=128))
                    v_sb = apool.tile([128, 4, W], f32, tag='v')
                    for mi in range(4):
                        vps = apspool.tile([128, W], f32, tag='ps')
                        for kt in range(4):
                            nc.tensor.matmul(
                                vps[:], wv_sb[:, kt, mi * 128:(mi + 1) * 128],
                                img_sb[:, kt, :], start=(kt == 0), stop=(kt == 3))
                        nc.scalar.copy(v_sb[:, mi, :], vps[:])
                    vT_sb = apool.tile([128, 4, H], bf16, tag='vt')
                    for ct in range(4):
                        tps = apspool.tile([128, H], f32, tag='ps')
                        for it in range(4):
                            nc.tensor.transpose(
                                tps[:, it * 128:(it + 1) * 128],
                                v_sb[:, it, ct * 128:(ct + 1) * 128], ident[:])
                        nc.scalar.copy(vT_sb[:, ct, :], tps[:])
                    o_sb = apool.tile([128, 4, W], f32, tag=f'o{ch}')
                    for mi in range(4):
                        ops = apspool.tile([128, W], f32, tag='ps')
                        for ct in range(4):
                            nc.tensor.matmul(
                                ops[:], vT_sb[:, ct, mi * 128:(mi + 1) * 128],
                                wh_sb[:, ct, :], start=(ct == 0), stop=(ct == 3))
                        nc.scalar.copy(o_sb[:, mi, :], ops[:])
                    Ot.append(o_sb)
                gray = apool.tile([128, 4, W], f32, tag='agray')
                for ch in range(C):
                    nc.vector.tensor_scalar(
                        Ot[ch][:], Ot[ch][:], scx_sb[:, ka, 0:1], scx_sb[:, ka, 1:2],
                        op0=mult, op1=add)
                    nc.vector.tensor_scalar(
                        Ot[ch][:], Ot[ch][:], scx_sb[:, ka, 2:3], scx_sb[:, ka, 5:6],
                        op0=amin, op1=amax)
                nc.vector.tensor_tensor(gray[:], Ot[0][:], Ot[1][:], op=add)
                nc.vector.tensor_tensor(gray[:], gray[:], Ot[2][:], op=add)
                nc.vector.tensor_scalar(gray[:], gray[:], scx_sb[:, ka, 4:5], None, op0=mult)
                if ax_cut[ka]:
                    cm_sb = apool.tile([128, W], f32, tag='cm')
                    rm_sb = apool.tile([128, 4], f32, tag='rm')
                    nc.sync.dma_start(cm_sb[:], d['cm'][ka])
                    nc.sync.dma_start(rm_sb[:], d['rm'][ka])
                for ch in range(C):
                    nc.vector.scalar_tensor_tensor(
                        Ot[ch][:], Ot[ch][:], scx_sb[:, ka, 3:4], gray[:],
                        op0=mult, op1=add)
                    nc.vector.tensor_scalar(
                        Ot[ch][:], Ot[ch][:], 1.0, -1.0, op0=amin, op1=amax)
                    if ax_cut[ka]:
                        n_sb = apool.tile([128, 4, W], f32, tag='n')
                        nc.scalar.dma_start(n_sb[:], d['nza'][ka, ch].rearrange("(t p) c -> p t c", p=128))
                        nc.vector.tensor_tensor(n_sb[:], n_sb[:], Ot[ch][:], op=mybir.AluOpType.subtract)
                        for tt in range(4):
                            nc.vector.tensor_tensor(n_sb[:, tt, :], n_sb[:, tt, :],
                                                    cm_sb[:], op=mult)
                            nc.vector.scalar_tensor_tensor(
                                Ot[ch][:, tt, :], n_sb[:, tt, :], rm_sb[:, tt:tt + 1],
                                Ot[ch][:, tt, :], op0=mult, op1=add)
                    nc.sync.dma_start(
                        d_oax[ka, ch].rearrange("(t p) c -> p t c", p=128), Ot[ch][:])
    nc.compile()
    return nc


# ---------------------------------------------------------------- entry
def kernel(**inputs):
    from concourse import bass_utils
    cores, meta, axinfo, plan = _host_prep(inputs)
    key = (plan['NRU'], plan['NG'], plan['NAX'], plan['plane_u'],
           plan['grp_cut'], plan['ax_cut'])
    if key not in _PROGRAM_CACHE:
        _PROGRAM_CACHE[key] = _build(plan)
    nc = _PROGRAM_CACHE[key]
    in_maps = [{k2: v for k2, v in c.items() if k2 != 'outmap'} for c in cores]
    res = bass_utils.run_bass_kernel_spmd(nc, in_maps, core_ids=list(range(NCORES)))
    out = np.zeros((B, C, H, W), np.float32)
    for c in range(NCORES):
        dump = res.results[c]['dump']
        for i, (s, t) in enumerate(cores[c]['outmap']):
            du = dump[i].reshape(8, 3, NI)
            blk = np.empty_like(du)
            blk[:, :, PERM] = du
            seg = blk.reshape(8, 3, BLK, BLK).transpose(1, 2, 0, 3).reshape(3, BLK, 256)
            b0 = t * 8
            bi, bj0 = b0 // GRID, b0 % GRID
            out[s, :, bi * BLK:(bi + 1) * BLK, bj0 * BLK:bj0 * BLK + 256] = seg
        oax = res.results[c]['oax']
        axl, axown = axinfo[c]
        for k2 in range(axown):
            out[axl[k2]] = oax[k2]
    return out
